# revision 9
# baseline (speedup 1.0000x reference)
"""2-layer GAT on 8 Trainium2 NeuronCores.

Sharding: nodes split 8 ways (12500/core); each core owns the edges whose
destination falls in its node range (dst-sorted, self-loops included), plus a
replicated copy of the layer's node-feature table (built distributed, then
AllGathered). Per 128-edge tile: one indirect-DMA gather of src rows, a
one-hot (iota==dst_local) matrix, PE-transposed to broadcast per-edge a_dst,
then exp(lrelu(a_src+a_dst)) and a one-hot scatter matmul accumulating
[numerator | denominator] per 128-node block in PSUM.
"""
import sys
sys.path.insert(0, "/opt/trn_rl_repo")
import numpy as np

import concourse.bass as bass
import concourse.tile as tile
from concourse import mybir
P = 128
N_CORES = 8
NEG_SLOPE = 0.2
F32 = mybir.dt.float32
I32 = mybir.dt.int32


def _split_multi_waits(nc):
    """This walrus build accepts at most one sem wait per instruction; hoist
    extras onto preceding same-engine NOPs (sequencers run in order)."""
    ctr = 0
    for bb in nc.main_func.blocks:
        new = []
        changed = False
        for ins in bb.instructions:
            si = ins.sync_info
            waits = list(si.on_wait) if si is not None and si.on_wait else []
            if len(waits) > 1:
                changed = True
                for w in waits[:-1]:
                    ctr += 1
                    new.append(mybir.InstNoOp(
                        name=f"wsplit_{ctr}", ins=[], outs=[], engine=ins.engine,
                        sync_info=mybir.SyncInfo(on_wait=[w], on_update=[])))
                si.on_wait = waits[-1:]
            new.append(ins)
        if changed:
            bb.instructions = new


def _host_prep(edge_index, n_nodes):
    """Integer-only preprocessing: shard by dst, sort, pad to 128-edge tiles
    per 128-node block; identical tile structure across cores (SPMD)."""
    npc = n_nodes // N_CORES               # nodes per core
    nb = (npc + P - 1) // P                # blocks per core
    npc_pad = nb * P
    src = np.concatenate([edge_index[0], np.arange(n_nodes, dtype=np.int64)])
    dst = np.concatenate([edge_index[1], np.arange(n_nodes, dtype=np.int64)])

    per_core = []
    counts_all = np.zeros((N_CORES, nb), np.int64)
    for c in range(N_CORES):
        sel = (dst // npc) == c
        ls = src[sel].astype(np.int64)
        ld = (dst[sel] - c * npc).astype(np.int64)
        order = np.argsort(ld, kind="stable")
        ls, ld = ls[order], ld[order]
        blk = ld // P
        counts_all[c] = np.bincount(blk, minlength=nb)
        per_core.append((ls, ld, blk))

    tiles_per_block = np.maximum(1, -(-counts_all.max(axis=0) // P))  # ceil
    cum_tiles = np.concatenate([[0], np.cumsum(tiles_per_block)])
    nt = int(cum_tiles[-1])

    src_idx_all, dst_loc_all = [], []
    for c in range(N_CORES):
        ls, ld, blk = per_core[c]
        starts = np.concatenate([[0], np.cumsum(counts_all[c])])
        rank = np.arange(len(ls)) - starts[blk]
        pos = P * cum_tiles[blk] + rank
        flat_src = np.zeros(nt * P, np.int32)        # pad: gather row 0
        flat_dl = np.full(nt * P, -1.0, np.float32)  # pad: no one-hot match
        # remap global node id -> padded table row (core*npc_pad + local)
        ls_core = ls // npc
        flat_src[pos] = (ls_core * npc_pad + (ls - ls_core * npc)).astype(np.int32)
        flat_dl[pos] = (ld - P * blk).astype(np.float32)
        src_idx_all.append(flat_src.reshape(nt, P).T.copy())   # [128, nt]
        dst_loc_all.append(flat_dl.reshape(nt, P).T.copy())    # [128, nt]

    bake = dict(npc=npc, nb=nb, npc_pad=npc_pad, nt=nt,
                tiles_per_block=[int(t) for t in tiles_per_block],
                cum_tiles=[int(t) for t in cum_tiles])
    return bake, src_idx_all, dst_loc_all


def _build_program(bake, nfeat, nhid, heads, nclass):
    """Emit the SPMD bass program (same for all cores)."""
    npc, nb, npc_pad, nt = bake["npc"], bake["nb"], bake["npc_pad"], bake["nt"]
    tpb, cumt = bake["tiles_per_block"], bake["cum_tiles"]
    hh = heads * nhid            # 128
    d1 = hh + 2 * heads          # 144: [h | a_src | a_dst]
    d2 = nclass + 2              # 42:  [h2 | a_src2 | a_dst2]
    ntab = N_CORES * npc_pad

    nc = bass.Bass()
    xT = nc.dram_tensor("xT", [nfeat, npc], F32, kind="ExternalInput")
    W1 = nc.dram_tensor("W1", [nfeat, hh], F32, kind="ExternalInput")
    W2 = nc.dram_tensor("W2", [hh, nclass], F32, kind="ExternalInput")
    asrc1 = nc.dram_tensor("asrc1", [P, hh], F32, kind="ExternalInput")
    adst1 = nc.dram_tensor("adst1", [P, hh], F32, kind="ExternalInput")
    asrc2 = nc.dram_tensor("asrc2", [P, nclass], F32, kind="ExternalInput")
    adst2 = nc.dram_tensor("adst2", [P, nclass], F32, kind="ExternalInput")
    b1r = nc.dram_tensor("b1r", [P, hh], F32, kind="ExternalInput")
    b2r = nc.dram_tensor("b2r", [P, nclass], F32, kind="ExternalInput")
    srcidx = nc.dram_tensor("srcidx", [P, nt], I32, kind="ExternalInput")
    iota_in = nc.dram_tensor("iota128", [P, P], F32, kind="ExternalInput")
    ident_in = nc.dram_tensor("ident128", [P, P], F32, kind="ExternalInput")
    dstloc = nc.dram_tensor("dstloc", [P, nt], F32, kind="ExternalInput")
    out = nc.dram_tensor("out", [npc, nclass], F32, kind="ExternalOutput")

    shard1 = nc.dram_tensor("shard1", [npc_pad, d1], F32)
    table1 = nc.dram_tensor("table1", [ntab, d1], F32, addr_space="Shared")
    shard2 = nc.dram_tensor("shard2", [npc_pad, d2], F32)
    table2 = nc.dram_tensor("table2", [ntab, d2], F32, addr_space="Shared")

    AF = mybir.ActivationFunctionType
    OP = mybir.AluOpType

    with tile.TileContext(nc) as tc:
        with tc.tile_pool(name="persist", bufs=1) as pp, \
             tc.tile_pool(name="work", bufs=3) as wp, \
             tc.tile_pool(name="psA", bufs=2, space="PSUM") as psA, \
             tc.tile_pool(name="psT", bufs=2, space="PSUM") as psT, \
             tc.tile_pool(name="psB", bufs=2, space="PSUM") as psB:

            # ---- constants (host-supplied)
            iota_f = pp.tile([P, P], F32)
            ident = pp.tile([P, P], F32)
            nc.sync.dma_start(iota_f[:], iota_in[:])
            nc.sync.dma_start(ident[:], ident_in[:])

            dl = pp.tile([P, nt], F32)
            si_t = pp.tile([P, nt], I32)
            nc.sync.dma_start(dl[:], dstloc[:])
            nc.sync.dma_start(si_t[:], srcidx[:])

            b1_t = pp.tile([P, hh], F32)
            b2_t = pp.tile([P, nclass], F32)
            nc.sync.dma_start(b1_t[:], b1r[:])
            nc.sync.dma_start(b2_t[:], b2r[:])

            # ---- phase A: W1_ext, h_ext shard, allgather table1
            w1_t = wp.tile([nfeat, hh], F32, tag="w1")
            nc.sync.dma_start(w1_t[:], W1[:])
            as1 = wp.tile([P, hh], F32, tag="as1")
            ad1 = wp.tile([P, hh], F32, tag="ad1")
            nc.sync.dma_start(as1[:], asrc1[:])
            nc.sync.dma_start(ad1[:], adst1[:])
            w1e = pp.tile([nfeat, d1], F32)
            nc.scalar.copy(w1e[:, 0:hh], w1_t[:])
            tmp = wp.tile([P, hh], F32, tag="tmpw")
            nc.vector.tensor_tensor(out=tmp[:], in0=w1_t[:], in1=as1[:], op=OP.mult)
            nc.vector.tensor_reduce(
                out=w1e[:, hh:hh + heads],
                in_=tmp[:].rearrange("p (h c) -> p h c", h=heads),
                axis=mybir.AxisListType.X, op=OP.add)
            nc.vector.tensor_tensor(out=tmp[:], in0=w1_t[:], in1=ad1[:], op=OP.mult)
            nc.vector.tensor_reduce(
                out=w1e[:, hh + heads:d1],
                in_=tmp[:].rearrange("p (h c) -> p h c", h=heads),
                axis=mybir.AxisListType.X, op=OP.add)

            adst1_own = pp.tile([P, nb, heads], F32)
            nc.vector.memset(adst1_own[:], 0.0)
            for b in range(nb):
                n0 = b * P
                cnt = min(P, npc - n0)
                xTb = wp.tile([nfeat, P], F32, tag="xTb")
                nc.sync.dma_start(xTb[:, 0:cnt], xT[:, n0:n0 + cnt])
                ps = psA.tile([P, d1], F32, tag="ps_a")
                nc.tensor.matmul(out=ps[:cnt, :], lhsT=xTb[:, 0:cnt],
                                 rhs=w1e[:], start=True, stop=True)
                stg = wp.tile([P, d1], F32, tag="stg1")
                if cnt < P:
                    nc.vector.memset(stg[:], 0.0)
                nc.scalar.copy(stg[:cnt, :], ps[:cnt, :])
                nc.vector.tensor_copy(adst1_own[:cnt, b, :],
                                      stg[:cnt, hh + heads:d1])
                nc.sync.dma_start(shard1[n0:n0 + P, :], stg[:])
            nc.gpsimd.collective_compute(
                "AllGather", OP.bypass,
                replica_groups=[list(range(N_CORES))],
                ins=[shard1[:]], outs=[table1[:]])

            # ---- edge phase helper
            def edge_layer(tab, dcols, hcols, nheads, hdim, adst_own, close_fn):
                """dcols: table row width; hcols: feature cols; per-tile:
                gather, one-hot, a_dst, alpha, scatter."""
                rhsw = hcols + nheads      # [v | alpha]
                for b in range(nb):
                    acc = psB.tile([P, rhsw], F32, tag="acc")
                    t0, t1 = cumt[b], cumt[b + 1]
                    t = t0
                    while t < t1:
                        nb_t = min(4, t1 - t)
                        G = wp.tile([P, 4, dcols], F32, tag="G")
                        for k in range(nb_t):
                            nc.gpsimd.indirect_dma_start(
                                out=G[:, k, :], out_offset=None, in_=tab[:],
                                in_offset=bass.IndirectOffsetOnAxis(
                                    ap=si_t[:, t + k:t + k + 1], axis=0))
                        oh = wp.tile([P, 4, P], F32, tag="oh")
                        nc.vector.tensor_tensor(
                            out=oh[:, 0:nb_t, :],
                            in0=iota_f[:].unsqueeze(1).to_broadcast(
                                [P, nb_t, P]),
                            in1=dl[:, t:t + nb_t].unsqueeze(2).to_broadcast(
                                [P, nb_t, P]),
                            op=OP.is_equal)
                        adp = psT.tile([P, 4 * nheads], F32, tag="adp")
                        for k in range(nb_t):
                            ohT_ps = psT.tile([P, P], F32, tag="ohT_ps")
                            nc.tensor.transpose(out=ohT_ps[:], in_=oh[:, k, :],
                                                identity=ident[:])
                            ohT = wp.tile([P, P], F32, tag="ohT")
                            nc.scalar.copy(ohT[:], ohT_ps[:])
                            nc.tensor.matmul(
                                out=adp[:, k * nheads:(k + 1) * nheads],
                                lhsT=ohT[:],
                                rhs=adst_own[:, b, :],
                                start=True, stop=True)
                        rhs = wp.tile([P, 4, rhsw], F32, tag="rhs")
                        s_t = wp.tile([P, 4 * nheads], F32, tag="s")
                        nc.vector.tensor_tensor(
                            out=s_t[:, 0:nb_t * nheads].rearrange(
                                "p (t h) -> p t h", h=nheads),
                            in0=G[:, 0:nb_t, hcols:hcols + nheads],
                            in1=adp[:, 0:nb_t * nheads].rearrange(
                                "p (t h) -> p t h", h=nheads),
                            op=OP.add)
                        nc.vector.scalar_tensor_tensor(
                            out=s_t[:, 0:nb_t * nheads],
                            in0=s_t[:, 0:nb_t * nheads], scalar=NEG_SLOPE,
                            in1=s_t[:, 0:nb_t * nheads],
                            op0=OP.mult, op1=OP.max)
                        nc.scalar.activation(
                            rhs[:, 0:nb_t, hcols:rhsw],
                            s_t[:, 0:nb_t * nheads].rearrange(
                                "p (t h) -> p t h", h=nheads),
                            AF.Exp)
                        nc.vector.tensor_tensor(
                            out=rhs[:, 0:nb_t, 0:hcols],
                            in0=G[:, 0:nb_t, 0:hcols],
                            in1=rhs[:, 0:nb_t, hcols:rhsw].unsqueeze(
                                3).to_broadcast([P, nb_t, nheads, hdim]),
                            op=OP.mult)
                        for k in range(nb_t):
                            nc.tensor.matmul(
                                out=acc[:], lhsT=oh[:, k, :], rhs=rhs[:, k, :],
                                start=(t + k == t0), stop=(t + k == t1 - 1),
                                skip_group_check=True)
                        t += nb_t
                    close_fn(b, acc)

            # ---- L1 close: normalize + bias + elu -> h1_own
            h1 = pp.tile([P, nb, hh], F32)
            nc.vector.memset(h1[:, nb - 1, :], 0.0)

            def close1(b, acc):
                d8 = wp.tile([P, heads], F32, tag="d8")
                nc.vector.tensor_scalar(out=d8[:], in0=acc[:, hh:hh + heads],
                                        scalar1=1e-16, scalar2=None, op0=OP.add)
                r8 = wp.tile([P, heads], F32, tag="r8")
                nc.vector.reciprocal(r8[:], d8[:])
                tt = wp.tile([P, hh], F32, tag="tt")
                nc.vector.tensor_tensor(
                    out=tt[:].rearrange("p (h c) -> p h c", h=heads),
                    in0=acc[:, 0:hh].rearrange("p (h c) -> p h c", h=heads),
                    in1=r8[:].unsqueeze(2).to_broadcast([P, heads, nhid]),
                    op=OP.mult)
                nc.vector.tensor_tensor(out=tt[:], in0=tt[:], in1=b1_t[:],
                                        op=OP.add)
                pos = wp.tile([P, hh], F32, tag="pos")
                neg = wp.tile([P, hh], F32, tag="neg")
                nc.vector.tensor_scalar(out=pos[:], in0=tt[:], scalar1=0.0,
                                        scalar2=None, op0=OP.max)
                nc.vector.tensor_scalar(out=neg[:], in0=tt[:], scalar1=0.0,
                                        scalar2=None, op0=OP.min)
                nc.scalar.activation(neg[:], neg[:], AF.Exp)
                nc.vector.scalar_tensor_tensor(
                    out=h1[:, b, :], in0=pos[:], scalar=-1.0, in1=neg[:],
                    op0=OP.add, op1=OP.add)

            edge_layer(table1, d1, hh, heads, nhid, adst1_own, close1)

            # ---- phase C: W2_ext, h2_ext shard, allgather table2
            w2_t = wp.tile([hh, nclass], F32, tag="w2")
            nc.sync.dma_start(w2_t[:], W2[:])
            as2 = wp.tile([P, nclass], F32, tag="as2")
            ad2 = wp.tile([P, nclass], F32, tag="ad2")
            nc.sync.dma_start(as2[:], asrc2[:])
            nc.sync.dma_start(ad2[:], adst2[:])
            w2e = pp.tile([hh, d2], F32)
            nc.scalar.copy(w2e[:, 0:nclass], w2_t[:])
            tmp2 = wp.tile([P, nclass], F32, tag="tmp2")
            nc.vector.tensor_tensor(out=tmp2[:], in0=w2_t[:], in1=as2[:], op=OP.mult)
            nc.vector.tensor_reduce(out=w2e[:, nclass:nclass + 1], in_=tmp2[:],
                                    axis=mybir.AxisListType.X, op=OP.add)
            nc.vector.tensor_tensor(out=tmp2[:], in0=w2_t[:], in1=ad2[:], op=OP.mult)
            nc.vector.tensor_reduce(out=w2e[:, nclass + 1:d2], in_=tmp2[:],
                                    axis=mybir.AxisListType.X, op=OP.add)

            adst2_own = pp.tile([P, nb, 1], F32)
            nc.vector.memset(adst2_own[:], 0.0)
            for b in range(nb):
                n0 = b * P
                cnt = min(P, npc - n0)
                tps = psA.tile([P, P], F32, tag="ps_a")
                nc.tensor.transpose(out=tps[:], in_=h1[:, b, :], identity=ident[:])
                h1T = wp.tile([P, P], F32, tag="h1T")
                nc.scalar.copy(h1T[:], tps[:])
                ps2 = psA.tile([P, d2], F32, tag="ps_a")
                nc.tensor.matmul(out=ps2[:cnt, :], lhsT=h1T[:, 0:cnt], rhs=w2e[:],
                                 start=True, stop=True)
                stg2 = wp.tile([P, d2], F32, tag="stg2")
                if cnt < P:
                    nc.vector.memset(stg2[:], 0.0)
                nc.scalar.copy(stg2[:cnt, :], ps2[:cnt, :])
                nc.vector.tensor_copy(adst2_own[:cnt, b, :],
                                      stg2[:cnt, nclass + 1:d2])
                nc.sync.dma_start(shard2[n0:n0 + P, :], stg2[:])
            nc.gpsimd.collective_compute(
                "AllGather", OP.bypass,
                replica_groups=[list(range(N_CORES))],
                ins=[shard2[:]], outs=[table2[:]])

            # ---- L2 close: log_softmax -> out
            def close2(b, acc):
                n0 = b * P
                cnt = min(P, npc - n0)
                d1_ = wp.tile([P, 1], F32, tag="d1_")
                nc.vector.tensor_scalar(out=d1_[:], in0=acc[:, nclass:nclass + 1],
                                        scalar1=1e-16, scalar2=None, op0=OP.add)
                r1 = wp.tile([P, 1], F32, tag="r1")
                nc.vector.reciprocal(r1[:], d1_[:])
                z = wp.tile([P, nclass], F32, tag="z")
                nc.vector.tensor_scalar(out=z[:], in0=acc[:, 0:nclass],
                                        scalar1=r1[:, 0:1], scalar2=None,
                                        op0=OP.mult)
                nc.vector.tensor_tensor(out=z[:], in0=z[:], in1=b2_t[:], op=OP.add)
                m = wp.tile([P, 1], F32, tag="m")
                nc.vector.tensor_reduce(out=m[:], in_=z[:],
                                        axis=mybir.AxisListType.X, op=OP.max)
                nc.vector.tensor_scalar(out=z[:], in0=z[:], scalar1=m[:, 0:1],
                                        scalar2=None, op0=OP.subtract)
                e = wp.tile([P, nclass], F32, tag="e")
                se = wp.tile([P, 1], F32, tag="se")
                nc.scalar.activation(e[:], z[:], AF.Exp, accum_out=se[:])
                lse = wp.tile([P, 1], F32, tag="lse")
                nc.scalar.activation(lse[:], se[:], AF.Ln)
                ob = wp.tile([P, nclass], F32, tag="ob")
                nc.vector.tensor_scalar(out=ob[:], in0=z[:], scalar1=lse[:, 0:1],
                                        scalar2=None, op0=OP.subtract)
                nc.sync.dma_start(out[n0:n0 + cnt, :], ob[:cnt, :])

            edge_layer(table2, d2, nclass, 1, nclass, adst2_own, close2)

    return nc


_CACHE = {}


def _get_program(bake, nfeat, nhid, heads, nclass):
    key = (bake["nt"], tuple(bake["tiles_per_block"]), nfeat, nhid, heads, nclass)
    if key not in _CACHE:
        nc = _build_program(bake, nfeat, nhid, heads, nclass)
        _split_multi_waits(nc)
        _CACHE[key] = nc
    return _CACHE[key]


def kernel(x, edge_index, W1, att_src1, att_dst1, b1, W2, att_src2, att_dst2, b2):
    from concourse.bass_utils import run_bass_kernel_spmd
    n_nodes, nfeat = x.shape
    heads, nhid = att_src1.shape[1], att_src1.shape[2]
    nclass = att_src2.shape[2]
    npc = n_nodes // N_CORES

    bake, src_idx_all, dst_loc_all = _host_prep(np.asarray(edge_index), n_nodes)
    nc = _get_program(bake, nfeat, nhid, heads, nclass)

    x = np.asarray(x, np.float32)
    in_maps = []
    for c in range(N_CORES):
        in_maps.append({
            "xT": np.ascontiguousarray(x[c * npc:(c + 1) * npc].T),
            "W1": np.asarray(W1, np.float32),
            "W2": np.asarray(W2, np.float32),
            "asrc1": np.tile(np.asarray(att_src1, np.float32).reshape(1, -1), (P, 1)),
            "adst1": np.tile(np.asarray(att_dst1, np.float32).reshape(1, -1), (P, 1)),
            "asrc2": np.tile(np.asarray(att_src2, np.float32).reshape(1, -1), (P, 1)),
            "adst2": np.tile(np.asarray(att_dst2, np.float32).reshape(1, -1), (P, 1)),
            "b1r": np.tile(np.asarray(b1, np.float32).reshape(1, -1), (P, 1)),
            "b2r": np.tile(np.asarray(b2, np.float32).reshape(1, -1), (P, 1)),
            "srcidx": src_idx_all[c],
            "iota128": np.tile(np.arange(P, dtype=np.float32), (P, 1)),
            "ident128": np.eye(P, dtype=np.float32),
            "dstloc": dst_loc_all[c],
        })
    res = run_bass_kernel_spmd(nc, in_maps, core_ids=list(range(N_CORES)))
    return np.concatenate([res.results[c]["out"] for c in range(N_CORES)], axis=0)


# revision 13
# speedup vs baseline: 561.3766x; 561.3766x over previous
"""2-layer GAT on 8 Trainium2 NeuronCores.

Sharding: nodes split 8 ways (12500/core); each core owns the edges whose
destination falls in its node range (dst-sorted, self-loops included), plus a
replicated copy of the layer's node-feature table (built distributed, then
AllGathered). Per 128-edge tile: one indirect-DMA gather of src rows, a
one-hot (iota==dst_local) matrix, PE-transposed to broadcast per-edge a_dst,
then exp(lrelu(a_src+a_dst)) and a one-hot scatter matmul accumulating
[numerator | denominator] per 128-node block in PSUM.
"""
import sys
sys.path.insert(0, "/opt/trn_rl_repo")
import numpy as np

import concourse.bass as bass
import concourse.tile as tile
from concourse import mybir
P = 128
N_CORES = 8
NEG_SLOPE = 0.2
F32 = mybir.dt.float32
F16 = mybir.dt.float16
I32 = mybir.dt.int32


def _split_multi_waits(nc):
    """This walrus build accepts at most one sem wait per instruction; hoist
    extras onto preceding same-engine NOPs (sequencers run in order)."""
    ctr = 0
    for bb in nc.main_func.blocks:
        new = []
        changed = False
        for ins in bb.instructions:
            si = ins.sync_info
            waits = list(si.on_wait) if si is not None and si.on_wait else []
            if len(waits) > 1:
                changed = True
                for w in waits[:-1]:
                    ctr += 1
                    new.append(mybir.InstNoOp(
                        name=f"wsplit_{ctr}", ins=[], outs=[], engine=ins.engine,
                        sync_info=mybir.SyncInfo(on_wait=[w], on_update=[])))
                si.on_wait = waits[-1:]
            new.append(ins)
        if changed:
            bb.instructions = new


def _host_prep(edge_index, n_nodes):
    """Integer-only preprocessing: shard by dst, sort, pad to 128-edge tiles
    per 128-node block; identical tile structure across cores (SPMD)."""
    npc = n_nodes // N_CORES               # nodes per core
    nb = (npc + P - 1) // P                # blocks per core
    npc_pad = nb * P
    src = np.concatenate([edge_index[0], np.arange(n_nodes, dtype=np.int64)])
    dst = np.concatenate([edge_index[1], np.arange(n_nodes, dtype=np.int64)])

    per_core = []
    counts_all = np.zeros((N_CORES, nb), np.int64)
    for c in range(N_CORES):
        sel = (dst // npc) == c
        ls = src[sel].astype(np.int64)
        ld = (dst[sel] - c * npc).astype(np.int64)
        order = np.argsort(ld, kind="stable")
        ls, ld = ls[order], ld[order]
        blk = ld // P
        counts_all[c] = np.bincount(blk, minlength=nb)
        per_core.append((ls, ld, blk))

    tiles_per_block = np.maximum(1, -(-counts_all.max(axis=0) // P))  # ceil
    cum_tiles = np.concatenate([[0], np.cumsum(tiles_per_block)])
    nt = int(cum_tiles[-1])

    src_idx_all, dst_loc_all = [], []
    for c in range(N_CORES):
        ls, ld, blk = per_core[c]
        starts = np.concatenate([[0], np.cumsum(counts_all[c])])
        rank = np.arange(len(ls)) - starts[blk]
        pos = P * cum_tiles[blk] + rank
        flat_src = np.zeros(nt * P, np.int32)        # pad: gather row 0
        flat_dl = np.full(nt * P, -1.0, np.float32)  # pad: no one-hot match
        # remap global node id -> padded table row (core*npc_pad + local)
        ls_core = ls // npc
        flat_src[pos] = (ls_core * npc_pad + (ls - ls_core * npc)).astype(np.int32)
        flat_dl[pos] = (ld - P * blk).astype(np.float32)
        src_idx_all.append(flat_src.reshape(nt, P).T.copy())   # [128, nt]
        dst_loc_all.append(flat_dl.reshape(nt, P).T.copy())    # [128, nt]

    bake = dict(npc=npc, nb=nb, npc_pad=npc_pad, nt=nt,
                tiles_per_block=[int(t) for t in tiles_per_block],
                cum_tiles=[int(t) for t in cum_tiles])
    return bake, src_idx_all, dst_loc_all


def _build_program(bake, nfeat, nhid, heads, nclass, do_l1=True, do_l2=True, do_cc=True):
    """Emit the SPMD bass program (same for all cores)."""
    npc, nb, npc_pad, nt = bake["npc"], bake["nb"], bake["npc_pad"], bake["nt"]
    tpb, cumt = bake["tiles_per_block"], bake["cum_tiles"]
    hh = heads * nhid            # 128
    d1 = hh + 2 * heads          # 144: [h | a_src | a_dst]
    d2 = nclass + 2              # 42:  [h2 | a_src2 | a_dst2]
    ntab = N_CORES * npc_pad

    nc = bass.Bass()
    xT = nc.dram_tensor("xT", [nfeat, npc], F32, kind="ExternalInput")
    W1 = nc.dram_tensor("W1", [nfeat, hh], F32, kind="ExternalInput")
    W2 = nc.dram_tensor("W2", [hh, nclass], F32, kind="ExternalInput")
    asrc1 = nc.dram_tensor("asrc1", [P, hh], F32, kind="ExternalInput")
    adst1 = nc.dram_tensor("adst1", [P, hh], F32, kind="ExternalInput")
    asrc2 = nc.dram_tensor("asrc2", [P, nclass], F32, kind="ExternalInput")
    adst2 = nc.dram_tensor("adst2", [P, nclass], F32, kind="ExternalInput")
    b1r = nc.dram_tensor("b1r", [P, hh], F32, kind="ExternalInput")
    b2r = nc.dram_tensor("b2r", [P, nclass], F32, kind="ExternalInput")
    srcidx = nc.dram_tensor("srcidx", [P, nt], I32, kind="ExternalInput")
    iota_in = nc.dram_tensor("iota128", [P, P], F32, kind="ExternalInput")
    ident_in = nc.dram_tensor("ident128", [P, P], F32, kind="ExternalInput")
    dstloc = nc.dram_tensor("dstloc", [P, nt], F32, kind="ExternalInput")
    out = nc.dram_tensor("out", [npc, nclass], F32, kind="ExternalOutput")

    shard1 = nc.dram_tensor("shard1", [npc_pad, d1], F16)
    table1 = nc.dram_tensor("table1", [ntab, d1], F16, addr_space="Shared")
    shard2 = nc.dram_tensor("shard2", [npc_pad, d2], F16)
    table2 = nc.dram_tensor("table2", [ntab, d2], F16, addr_space="Shared")

    AF = mybir.ActivationFunctionType
    OP = mybir.AluOpType

    with tile.TileContext(nc) as tc:
        with tc.tile_pool(name="persist", bufs=1) as pp, \
             tc.tile_pool(name="work", bufs=3) as wp, \
             tc.tile_pool(name="psA", bufs=2, space="PSUM") as psA, \
             tc.tile_pool(name="psT", bufs=2, space="PSUM") as psT, \
             tc.tile_pool(name="psB", bufs=2, space="PSUM") as psB:

            # ---- constants (host-supplied)
            iota_f = pp.tile([P, P], F32)
            ident = pp.tile([P, P], F32)
            nc.sync.dma_start(iota_f[:], iota_in[:])
            nc.sync.dma_start(ident[:], ident_in[:])
            ident16 = pp.tile([P, P], F16)
            nc.vector.tensor_copy(ident16[:], ident[:])

            dl = pp.tile([P, nt], F32)
            si_t = pp.tile([P, nt], I32)
            nc.sync.dma_start(dl[:], dstloc[:])
            nc.sync.dma_start(si_t[:], srcidx[:])

            b1_t = pp.tile([P, hh], F32)
            b2_t = pp.tile([P, nclass], F32)
            nc.sync.dma_start(b1_t[:], b1r[:])
            nc.sync.dma_start(b2_t[:], b2r[:])

            # ---- phase A: W1_ext, h_ext shard, allgather table1
            w1_t = wp.tile([nfeat, hh], F32, tag="w1")
            nc.sync.dma_start(w1_t[:], W1[:])
            as1 = wp.tile([P, hh], F32, tag="as1")
            ad1 = wp.tile([P, hh], F32, tag="ad1")
            nc.sync.dma_start(as1[:], asrc1[:])
            nc.sync.dma_start(ad1[:], adst1[:])
            w1e = pp.tile([nfeat, d1], F32)
            nc.scalar.copy(w1e[:, 0:hh], w1_t[:])
            tmp = wp.tile([P, hh], F32, tag="tmpw")
            nc.vector.tensor_tensor(out=tmp[:], in0=w1_t[:], in1=as1[:], op=OP.mult)
            nc.vector.tensor_reduce(
                out=w1e[:, hh:hh + heads],
                in_=tmp[:].rearrange("p (h c) -> p h c", h=heads),
                axis=mybir.AxisListType.X, op=OP.add)
            nc.vector.tensor_tensor(out=tmp[:], in0=w1_t[:], in1=ad1[:], op=OP.mult)
            nc.vector.tensor_reduce(
                out=w1e[:, hh + heads:d1],
                in_=tmp[:].rearrange("p (h c) -> p h c", h=heads),
                axis=mybir.AxisListType.X, op=OP.add)

            adst1_own = pp.tile([P, nb, heads], F16)
            nc.vector.memset(adst1_own[:], 0.0)
            for b in range(nb):
                n0 = b * P
                cnt = min(P, npc - n0)
                xTb = wp.tile([nfeat, P], F32, tag="xTb")
                nc.sync.dma_start(xTb[:, 0:cnt], xT[:, n0:n0 + cnt])
                ps = psA.tile([P, d1], F32, tag="ps_a")
                nc.tensor.matmul(out=ps[:cnt, :], lhsT=xTb[:, 0:cnt],
                                 rhs=w1e[:], start=True, stop=True)
                stg = wp.tile([P, d1], F16, tag="stg1")
                if cnt < P:
                    nc.vector.memset(stg[:], 0.0)
                nc.scalar.copy(stg[:cnt, :], ps[:cnt, :])
                nc.vector.tensor_copy(adst1_own[:cnt, b, :],
                                      stg[:cnt, hh + heads:d1])
                nc.sync.dma_start(shard1[n0:n0 + P, :], stg[:])
            if do_cc:
                nc.gpsimd.collective_compute(
                    "AllGather", OP.bypass,
                    replica_groups=[list(range(N_CORES))],
                    ins=[shard1[:]], outs=[table1[:]])
            else:
                nc.sync.dma_start(table1[0:npc_pad, :], shard1[:])

            # ---- edge phase helper
            def edge_layer(tab, dcols, hcols, nheads, hdim, adst_own, close_fn):
                """dcols: table row width; hcols: feature cols; per-tile:
                gather, one-hot, a_dst, alpha, scatter."""
                rhsw = hcols + nheads      # [v | alpha]
                for b in range(nb):
                    acc = psB.tile([P, rhsw], F32, tag="acc")
                    t0, t1 = cumt[b], cumt[b + 1]
                    t = t0
                    while t < t1:
                        nb_t = min(4, t1 - t)
                        G = wp.tile([P, 4, dcols], F16, tag="G")
                        for k in range(nb_t):
                            nc.gpsimd.indirect_dma_start(
                                out=G[:, k, :], out_offset=None, in_=tab[:],
                                in_offset=bass.IndirectOffsetOnAxis(
                                    ap=si_t[:, t + k:t + k + 1], axis=0))
                        oh = wp.tile([P, 4, P], F16, tag="oh")
                        nc.vector.tensor_tensor(
                            out=oh[:, 0:nb_t, :],
                            in0=iota_f[:].unsqueeze(1).to_broadcast(
                                [P, nb_t, P]),
                            in1=dl[:, t:t + nb_t].unsqueeze(2).to_broadcast(
                                [P, nb_t, P]),
                            op=OP.is_equal)
                        adp = psT.tile([P, 4 * nheads], F32, tag="adp")
                        for k in range(nb_t):
                            ohT_ps = psT.tile([P, P], F16, tag="ohT_ps")
                            nc.tensor.transpose(out=ohT_ps[:], in_=oh[:, k, :],
                                                identity=ident16[:])
                            ohT = wp.tile([P, P], F16, tag="ohT")
                            nc.scalar.copy(ohT[:], ohT_ps[:])
                            nc.tensor.matmul(
                                out=adp[:, k * nheads:(k + 1) * nheads],
                                lhsT=ohT[:],
                                rhs=adst_own[:, b, :],
                                start=True, stop=True)
                        rhs = wp.tile([P, 4, rhsw], F16, tag="rhs")
                        s_t = wp.tile([P, 4 * nheads], F32, tag="s")
                        nc.vector.tensor_tensor(
                            out=s_t[:, 0:nb_t * nheads].rearrange(
                                "p (t h) -> p t h", h=nheads),
                            in0=G[:, 0:nb_t, hcols:hcols + nheads],
                            in1=adp[:, 0:nb_t * nheads].rearrange(
                                "p (t h) -> p t h", h=nheads),
                            op=OP.add)
                        nc.vector.scalar_tensor_tensor(
                            out=s_t[:, 0:nb_t * nheads],
                            in0=s_t[:, 0:nb_t * nheads], scalar=NEG_SLOPE,
                            in1=s_t[:, 0:nb_t * nheads],
                            op0=OP.mult, op1=OP.max)
                        nc.scalar.activation(
                            rhs[:, 0:nb_t, hcols:rhsw],
                            s_t[:, 0:nb_t * nheads].rearrange(
                                "p (t h) -> p t h", h=nheads),
                            AF.Exp)
                        nc.vector.tensor_tensor(
                            out=rhs[:, 0:nb_t, 0:hcols],
                            in0=G[:, 0:nb_t, 0:hcols],
                            in1=rhs[:, 0:nb_t, hcols:rhsw].unsqueeze(
                                3).to_broadcast([P, nb_t, nheads, hdim]),
                            op=OP.mult)
                        for k in range(nb_t):
                            nc.tensor.matmul(
                                out=acc[:], lhsT=oh[:, k, :], rhs=rhs[:, k, :],
                                start=(t + k == t0), stop=(t + k == t1 - 1),
                                skip_group_check=True)
                        t += nb_t
                    close_fn(b, acc)

            # ---- L1 close: normalize + bias + elu -> h1_own
            h1 = pp.tile([P, nb, hh], F32)
            nc.vector.memset(h1[:, nb - 1, :], 0.0)

            def close1(b, acc):
                d8 = wp.tile([P, heads], F32, tag="d8")
                nc.vector.tensor_scalar(out=d8[:], in0=acc[:, hh:hh + heads],
                                        scalar1=1e-16, scalar2=None, op0=OP.add)
                r8 = wp.tile([P, heads], F32, tag="r8")
                nc.vector.reciprocal(r8[:], d8[:])
                tt = wp.tile([P, hh], F32, tag="tt")
                nc.vector.tensor_tensor(
                    out=tt[:].rearrange("p (h c) -> p h c", h=heads),
                    in0=acc[:, 0:hh].rearrange("p (h c) -> p h c", h=heads),
                    in1=r8[:].unsqueeze(2).to_broadcast([P, heads, nhid]),
                    op=OP.mult)
                nc.vector.tensor_tensor(out=tt[:], in0=tt[:], in1=b1_t[:],
                                        op=OP.add)
                pos = wp.tile([P, hh], F32, tag="pos")
                neg = wp.tile([P, hh], F32, tag="neg")
                nc.vector.tensor_scalar(out=pos[:], in0=tt[:], scalar1=0.0,
                                        scalar2=None, op0=OP.max)
                nc.vector.tensor_scalar(out=neg[:], in0=tt[:], scalar1=0.0,
                                        scalar2=None, op0=OP.min)
                nc.scalar.activation(neg[:], neg[:], AF.Exp)
                nc.vector.scalar_tensor_tensor(
                    out=h1[:, b, :], in0=pos[:], scalar=-1.0, in1=neg[:],
                    op0=OP.add, op1=OP.add)

            if do_l1:
                edge_layer(table1, d1, hh, heads, nhid, adst1_own, close1)
            else:
                nc.vector.memset(h1[:], 0.0)

            # ---- phase C: W2_ext, h2_ext shard, allgather table2
            w2_t = wp.tile([hh, nclass], F32, tag="w2")
            nc.sync.dma_start(w2_t[:], W2[:])
            as2 = wp.tile([P, nclass], F32, tag="as2")
            ad2 = wp.tile([P, nclass], F32, tag="ad2")
            nc.sync.dma_start(as2[:], asrc2[:])
            nc.sync.dma_start(ad2[:], adst2[:])
            w2e = pp.tile([hh, d2], F32)
            nc.scalar.copy(w2e[:, 0:nclass], w2_t[:])
            tmp2 = wp.tile([P, nclass], F32, tag="tmp2")
            nc.vector.tensor_tensor(out=tmp2[:], in0=w2_t[:], in1=as2[:], op=OP.mult)
            nc.vector.tensor_reduce(out=w2e[:, nclass:nclass + 1], in_=tmp2[:],
                                    axis=mybir.AxisListType.X, op=OP.add)
            nc.vector.tensor_tensor(out=tmp2[:], in0=w2_t[:], in1=ad2[:], op=OP.mult)
            nc.vector.tensor_reduce(out=w2e[:, nclass + 1:d2], in_=tmp2[:],
                                    axis=mybir.AxisListType.X, op=OP.add)

            adst2_own = pp.tile([P, nb, 1], F16)
            nc.vector.memset(adst2_own[:], 0.0)
            for b in range(nb):
                n0 = b * P
                cnt = min(P, npc - n0)
                tps = psA.tile([P, P], F32, tag="ps_a")
                nc.tensor.transpose(out=tps[:], in_=h1[:, b, :], identity=ident[:])
                h1T = wp.tile([P, P], F32, tag="h1T")
                nc.scalar.copy(h1T[:], tps[:])
                ps2 = psA.tile([P, d2], F32, tag="ps_a")
                nc.tensor.matmul(out=ps2[:cnt, :], lhsT=h1T[:, 0:cnt], rhs=w2e[:],
                                 start=True, stop=True)
                stg2 = wp.tile([P, d2], F16, tag="stg2")
                if cnt < P:
                    nc.vector.memset(stg2[:], 0.0)
                nc.scalar.copy(stg2[:cnt, :], ps2[:cnt, :])
                nc.vector.tensor_copy(adst2_own[:cnt, b, :],
                                      stg2[:cnt, nclass + 1:d2])
                nc.sync.dma_start(shard2[n0:n0 + P, :], stg2[:])
            if do_cc:
                nc.gpsimd.collective_compute(
                    "AllGather", OP.bypass,
                    replica_groups=[list(range(N_CORES))],
                    ins=[shard2[:]], outs=[table2[:]])
            else:
                nc.sync.dma_start(table2[0:npc_pad, :], shard2[:])

            # ---- L2 close: log_softmax -> out
            def close2(b, acc):
                n0 = b * P
                cnt = min(P, npc - n0)
                d1_ = wp.tile([P, 1], F32, tag="d1_")
                nc.vector.tensor_scalar(out=d1_[:], in0=acc[:, nclass:nclass + 1],
                                        scalar1=1e-16, scalar2=None, op0=OP.add)
                r1 = wp.tile([P, 1], F32, tag="r1")
                nc.vector.reciprocal(r1[:], d1_[:])
                z = wp.tile([P, nclass], F32, tag="z")
                nc.vector.tensor_scalar(out=z[:], in0=acc[:, 0:nclass],
                                        scalar1=r1[:, 0:1], scalar2=None,
                                        op0=OP.mult)
                nc.vector.tensor_tensor(out=z[:], in0=z[:], in1=b2_t[:], op=OP.add)
                m = wp.tile([P, 1], F32, tag="m")
                nc.vector.tensor_reduce(out=m[:], in_=z[:],
                                        axis=mybir.AxisListType.X, op=OP.max)
                nc.vector.tensor_scalar(out=z[:], in0=z[:], scalar1=m[:, 0:1],
                                        scalar2=None, op0=OP.subtract)
                e = wp.tile([P, nclass], F32, tag="e")
                se = wp.tile([P, 1], F32, tag="se")
                nc.scalar.activation(e[:], z[:], AF.Exp, accum_out=se[:])
                lse = wp.tile([P, 1], F32, tag="lse")
                nc.scalar.activation(lse[:], se[:], AF.Ln)
                ob = wp.tile([P, nclass], F32, tag="ob")
                nc.vector.tensor_scalar(out=ob[:], in0=z[:], scalar1=lse[:, 0:1],
                                        scalar2=None, op0=OP.subtract)
                nc.sync.dma_start(out[n0:n0 + cnt, :], ob[:cnt, :])

            if do_l2:
                edge_layer(table2, d2, nclass, 1, nclass, adst2_own, close2)
            else:
                for b in range(nb):
                    n0 = b * P
                    cnt = min(P, npc - n0)
                    zb = wp.tile([P, nclass], F32, tag="zb")
                    nc.vector.memset(zb[:], 0.0)
                    nc.sync.dma_start(out[n0:n0 + cnt, :], zb[:cnt, :])

    return nc


_CACHE = {}


def _get_program(bake, nfeat, nhid, heads, nclass, do_l1=True, do_l2=True, do_cc=True):
    key = (bake["nt"], tuple(bake["tiles_per_block"]), nfeat, nhid, heads, nclass,
           do_l1, do_l2, do_cc)
    if key not in _CACHE:
        nc = _build_program(bake, nfeat, nhid, heads, nclass, do_l1, do_l2, do_cc)
        _split_multi_waits(nc)
        _CACHE[key] = nc
    return _CACHE[key]


def kernel(x, edge_index, W1, att_src1, att_dst1, b1, W2, att_src2, att_dst2, b2):
    from concourse.bass_utils import run_bass_kernel_spmd
    n_nodes, nfeat = x.shape
    heads, nhid = att_src1.shape[1], att_src1.shape[2]
    nclass = att_src2.shape[2]
    npc = n_nodes // N_CORES

    bake, src_idx_all, dst_loc_all = _host_prep(np.asarray(edge_index), n_nodes)
    nc = _get_program(bake, nfeat, nhid, heads, nclass)

    x = np.asarray(x, np.float32)
    in_maps = []
    for c in range(N_CORES):
        in_maps.append({
            "xT": np.ascontiguousarray(x[c * npc:(c + 1) * npc].T),
            "W1": np.asarray(W1, np.float32),
            "W2": np.asarray(W2, np.float32),
            "asrc1": np.tile(np.asarray(att_src1, np.float32).reshape(1, -1), (P, 1)),
            "adst1": np.tile(np.asarray(att_dst1, np.float32).reshape(1, -1), (P, 1)),
            "asrc2": np.tile(np.asarray(att_src2, np.float32).reshape(1, -1), (P, 1)),
            "adst2": np.tile(np.asarray(att_dst2, np.float32).reshape(1, -1), (P, 1)),
            "b1r": np.tile(np.asarray(b1, np.float32).reshape(1, -1), (P, 1)),
            "b2r": np.tile(np.asarray(b2, np.float32).reshape(1, -1), (P, 1)),
            "srcidx": src_idx_all[c],
            "iota128": np.tile(np.arange(P, dtype=np.float32), (P, 1)),
            "ident128": np.eye(P, dtype=np.float32),
            "dstloc": dst_loc_all[c],
        })
    res = run_bass_kernel_spmd(nc, in_maps, core_ids=list(range(N_CORES)))
    return np.concatenate([res.results[c]["out"] for c in range(N_CORES)], axis=0)


# revision 16
# speedup vs baseline: 663.9333x; 1.1827x over previous
"""2-layer GAT (PyG GATConv semantics) on 8 Trainium2 NeuronCores.

Sharding (graph partitioning per the hint): nodes split 8 ways (12500/core);
each core owns the edges whose destination falls in its node range
(dst-sorted, self-loops appended, padded to 128-edge tiles per 128-node
block; identical tile structure across cores so one SPMD program serves all).

Per layer:
  1. Each core computes its shard of the node table
     [h | a_src | a_dst] = x @ [W | W*att_src | W*att_dst] and the shards are
     AllGathered into a replicated fp16 table (the "halo exchange").
  2. Per 128-edge tile: one indirect-DMA gather of the 128 src rows
     (the only gather primitive that works on this toolchain; ~1.2 us each on
     the gpsimd SWDGE engine - the kernel's bottleneck), a one-hot
     (iota == dst_local) matrix on DVE, PE-transpose of the one-hot + a tiny
     matmul to broadcast per-edge a_dst from the destination-side columns,
     exp(max(s, 0.2 s)) without max-subtraction (values are small, safe),
     then a one-hot scatter matmul accumulating [alpha*h_src | alpha] =
     [numerator | denominator] per 128-node block in PSUM.
  3. Block close: numer/denom (+1e-16), bias, elu (layer 1) or log_softmax
     (layer 2, output shard).

Toolchain workarounds: this walrus accepts max ONE sem wait/instruction
(_split_multi_waits hoists extras onto NOPs); gpsimd custom ISA ops
(dma_gather/partition_broadcast/...) are broken here; indirect_dma_start
only works with [128,1] offset lists.

Measured on the 8-core axon TRN2: ~6.3 ms HW time (wall minus matched-arg
trivial-kernel wall), rel err ~1e-4 vs the fp32 reference.
"""
import sys
sys.path.insert(0, "/opt/trn_rl_repo")
import numpy as np

import concourse.bass as bass
import concourse.tile as tile
from concourse import mybir
P = 128
N_CORES = 8
NEG_SLOPE = 0.2
G_BUFS = 3
SCRATCH = 16384
F32 = mybir.dt.float32
F16 = mybir.dt.float16
I32 = mybir.dt.int32


def _split_multi_waits(nc):
    """This walrus build accepts at most one sem wait per instruction; hoist
    extras onto preceding same-engine NOPs (sequencers run in order)."""
    ctr = 0
    for bb in nc.main_func.blocks:
        new = []
        changed = False
        for ins in bb.instructions:
            si = ins.sync_info
            waits = list(si.on_wait) if si is not None and si.on_wait else []
            if len(waits) > 1:
                changed = True
                for w in waits[:-1]:
                    ctr += 1
                    new.append(mybir.InstNoOp(
                        name=f"wsplit_{ctr}", ins=[], outs=[], engine=ins.engine,
                        sync_info=mybir.SyncInfo(on_wait=[w], on_update=[])))
                si.on_wait = waits[-1:]
            new.append(ins)
        if changed:
            bb.instructions = new


def _host_prep(edge_index, n_nodes):
    """Integer-only preprocessing: shard by dst, sort, pad to 128-edge tiles
    per 128-node block; identical tile structure across cores (SPMD)."""
    npc = n_nodes // N_CORES               # nodes per core
    nb = (npc + P - 1) // P                # blocks per core
    npc_pad = nb * P
    src = np.concatenate([edge_index[0], np.arange(n_nodes, dtype=np.int64)])
    dst = np.concatenate([edge_index[1], np.arange(n_nodes, dtype=np.int64)])

    per_core = []
    counts_all = np.zeros((N_CORES, nb), np.int64)
    for c in range(N_CORES):
        sel = (dst // npc) == c
        ls = src[sel].astype(np.int64)
        ld = (dst[sel] - c * npc).astype(np.int64)
        order = np.argsort(ld, kind="stable")
        ls, ld = ls[order], ld[order]
        blk = ld // P
        counts_all[c] = np.bincount(blk, minlength=nb)
        per_core.append((ls, ld, blk))

    tiles_per_block = np.maximum(1, -(-counts_all.max(axis=0) // P))  # ceil
    cum_tiles = np.concatenate([[0], np.cumsum(tiles_per_block)])
    nt = int(cum_tiles[-1])

    src_idx_all, dst_loc_all = [], []
    for c in range(N_CORES):
        ls, ld, blk = per_core[c]
        starts = np.concatenate([[0], np.cumsum(counts_all[c])])
        rank = np.arange(len(ls)) - starts[blk]
        pos = P * cum_tiles[blk] + rank
        flat_src = np.zeros(nt * P, np.int32)        # pad: gather row 0
        flat_dl = np.full(nt * P, -1.0, np.float32)  # pad: no one-hot match
        # remap global node id -> padded table row (core*npc_pad + local)
        ls_core = ls // npc
        flat_src[pos] = (ls_core * npc_pad + (ls - ls_core * npc)).astype(np.int32)
        flat_dl[pos] = (ld - P * blk).astype(np.float32)
        src_idx_all.append(flat_src.reshape(nt, P).T.copy())   # [128, nt]
        dst_loc_all.append(flat_dl.reshape(nt, P).T.copy())    # [128, nt]

    bake = dict(npc=npc, nb=nb, npc_pad=npc_pad, nt=nt,
                tiles_per_block=[int(t) for t in tiles_per_block],
                cum_tiles=[int(t) for t in cum_tiles])
    return bake, src_idx_all, dst_loc_all


def _build_program(bake, nfeat, nhid, heads, nclass, do_l1=True, do_l2=True, do_cc=True):
    """Emit the SPMD bass program (same for all cores)."""
    npc, nb, npc_pad, nt = bake["npc"], bake["nb"], bake["npc_pad"], bake["nt"]
    cumt = bake["cum_tiles"]
    hh = heads * nhid            # 128
    d1 = hh + 2 * heads          # 144: [h | a_src | a_dst]
    d2 = nclass + 2              # 42:  [h2 | a_src2 | a_dst2]
    ntab = N_CORES * npc_pad

    nc = bass.Bass(dynamic_dma_scratch_size=SCRATCH)
    xT = nc.dram_tensor("xT", [nfeat, npc], F32, kind="ExternalInput")
    W1 = nc.dram_tensor("W1", [nfeat, hh], F32, kind="ExternalInput")
    W2 = nc.dram_tensor("W2", [hh, nclass], F32, kind="ExternalInput")
    asrc1 = nc.dram_tensor("asrc1", [P, hh], F32, kind="ExternalInput")
    adst1 = nc.dram_tensor("adst1", [P, hh], F32, kind="ExternalInput")
    asrc2 = nc.dram_tensor("asrc2", [P, nclass], F32, kind="ExternalInput")
    adst2 = nc.dram_tensor("adst2", [P, nclass], F32, kind="ExternalInput")
    b1r = nc.dram_tensor("b1r", [P, hh], F32, kind="ExternalInput")
    b2r = nc.dram_tensor("b2r", [P, nclass], F32, kind="ExternalInput")
    srcidx = nc.dram_tensor("srcidx", [P, nt], I32, kind="ExternalInput")
    iota_in = nc.dram_tensor("iota128", [P, P], F32, kind="ExternalInput")
    ident_in = nc.dram_tensor("ident128", [P, P], F32, kind="ExternalInput")
    dstloc = nc.dram_tensor("dstloc", [P, nt], F32, kind="ExternalInput")
    out = nc.dram_tensor("out", [npc, nclass], F32, kind="ExternalOutput")

    shard1 = nc.dram_tensor("shard1", [npc_pad, d1], F16)
    table1 = nc.dram_tensor("table1", [ntab, d1], F16, addr_space="Shared")
    shard2 = nc.dram_tensor("shard2", [npc_pad, d2], F16)
    table2 = nc.dram_tensor("table2", [ntab, d2], F16, addr_space="Shared")

    AF = mybir.ActivationFunctionType
    OP = mybir.AluOpType

    with tile.TileContext(nc) as tc:
        with tc.tile_pool(name="persist", bufs=1) as pp, \
             tc.tile_pool(name="work", bufs=3) as wp, \
             tc.tile_pool(name="gpool", bufs=G_BUFS) as gp, \
             tc.tile_pool(name="psA", bufs=2, space="PSUM") as psA, \
             tc.tile_pool(name="psT", bufs=2, space="PSUM") as psT, \
             tc.tile_pool(name="psB", bufs=2, space="PSUM") as psB:

            # ---- constants (host-supplied)
            iota_f = pp.tile([P, P], F32)
            ident = pp.tile([P, P], F32)
            nc.sync.dma_start(iota_f[:], iota_in[:])
            nc.sync.dma_start(ident[:], ident_in[:])
            ident16 = pp.tile([P, P], F16)
            nc.vector.tensor_copy(ident16[:], ident[:])

            dl = pp.tile([P, nt], F32)
            si_t = pp.tile([P, nt], I32)
            nc.sync.dma_start(dl[:], dstloc[:])
            nc.sync.dma_start(si_t[:], srcidx[:])

            b1_t = pp.tile([P, hh], F32)
            b2_t = pp.tile([P, nclass], F32)
            nc.sync.dma_start(b1_t[:], b1r[:])
            nc.sync.dma_start(b2_t[:], b2r[:])

            # ---- phase A: W1_ext, h_ext shard, allgather table1
            w1_t = wp.tile([nfeat, hh], F32, tag="w1")
            nc.sync.dma_start(w1_t[:], W1[:])
            as1 = wp.tile([P, hh], F32, tag="as1")
            ad1 = wp.tile([P, hh], F32, tag="ad1")
            nc.sync.dma_start(as1[:], asrc1[:])
            nc.sync.dma_start(ad1[:], adst1[:])
            w1e = pp.tile([nfeat, d1], F32)
            nc.scalar.copy(w1e[:, 0:hh], w1_t[:])
            tmp = wp.tile([P, hh], F32, tag="tmpw")
            nc.vector.tensor_tensor(out=tmp[:], in0=w1_t[:], in1=as1[:], op=OP.mult)
            nc.vector.tensor_reduce(
                out=w1e[:, hh:hh + heads],
                in_=tmp[:].rearrange("p (h c) -> p h c", h=heads),
                axis=mybir.AxisListType.X, op=OP.add)
            nc.vector.tensor_tensor(out=tmp[:], in0=w1_t[:], in1=ad1[:], op=OP.mult)
            nc.vector.tensor_reduce(
                out=w1e[:, hh + heads:d1],
                in_=tmp[:].rearrange("p (h c) -> p h c", h=heads),
                axis=mybir.AxisListType.X, op=OP.add)

            adst1_own = pp.tile([P, nb, heads], F16)
            nc.vector.memset(adst1_own[:], 0.0)
            for b in range(nb):
                n0 = b * P
                cnt = min(P, npc - n0)
                xTb = wp.tile([nfeat, P], F32, tag="xTb")
                nc.sync.dma_start(xTb[:, 0:cnt], xT[:, n0:n0 + cnt])
                ps = psA.tile([P, d1], F32, tag="ps_a")
                nc.tensor.matmul(out=ps[:cnt, :], lhsT=xTb[:, 0:cnt],
                                 rhs=w1e[:], start=True, stop=True)
                stg = wp.tile([P, d1], F16, tag="stg1")
                if cnt < P:
                    nc.vector.memset(stg[:], 0.0)
                nc.scalar.copy(stg[:cnt, :], ps[:cnt, :])
                nc.vector.tensor_copy(adst1_own[:cnt, b, :],
                                      stg[:cnt, hh + heads:d1])
                nc.sync.dma_start(shard1[n0:n0 + P, :], stg[:])
            if do_cc:
                nc.gpsimd.collective_compute(
                    "AllGather", OP.bypass,
                    replica_groups=[list(range(N_CORES))],
                    ins=[shard1[:]], outs=[table1[:]])
            else:
                nc.sync.dma_start(table1[0:npc_pad, :], shard1[:])

            # ---- edge phase helper
            def edge_layer(tab, dcols, hcols, nheads, hdim, adst_own, close_fn):
                """dcols: table row width; hcols: feature cols; per-tile:
                gather, one-hot, a_dst, alpha, scatter."""
                rhsw = hcols + nheads      # [v | alpha]
                for b in range(nb):
                    acc = psB.tile([P, rhsw], F32, tag="acc")
                    t0, t1 = cumt[b], cumt[b + 1]
                    t = t0
                    while t < t1:
                        nb_t = min(4, t1 - t)
                        G = gp.tile([P, 4, dcols], F16, tag="G")
                        for k in range(nb_t):
                            nc.gpsimd.indirect_dma_start(
                                out=G[:, k, :], out_offset=None, in_=tab[:],
                                in_offset=bass.IndirectOffsetOnAxis(
                                    ap=si_t[:, t + k:t + k + 1], axis=0))
                        oh = wp.tile([P, 4, P], F16, tag="oh")
                        nc.vector.tensor_tensor(
                            out=oh[:, 0:nb_t, :],
                            in0=iota_f[:].unsqueeze(1).to_broadcast(
                                [P, nb_t, P]),
                            in1=dl[:, t:t + nb_t].unsqueeze(2).to_broadcast(
                                [P, nb_t, P]),
                            op=OP.is_equal)
                        adp = psT.tile([P, 4 * nheads], F32, tag="adp")
                        for k in range(nb_t):
                            ohT_ps = psT.tile([P, P], F16, tag="ohT_ps")
                            nc.tensor.transpose(out=ohT_ps[:], in_=oh[:, k, :],
                                                identity=ident16[:])
                            ohT = wp.tile([P, P], F16, tag="ohT")
                            nc.scalar.copy(ohT[:], ohT_ps[:])
                            nc.tensor.matmul(
                                out=adp[:, k * nheads:(k + 1) * nheads],
                                lhsT=ohT[:],
                                rhs=adst_own[:, b, :],
                                start=True, stop=True)
                        rhs = wp.tile([P, 4, rhsw], F16, tag="rhs")
                        s_t = wp.tile([P, 4 * nheads], F32, tag="s")
                        nc.vector.tensor_tensor(
                            out=s_t[:, 0:nb_t * nheads].rearrange(
                                "p (t h) -> p t h", h=nheads),
                            in0=G[:, 0:nb_t, hcols:hcols + nheads],
                            in1=adp[:, 0:nb_t * nheads].rearrange(
                                "p (t h) -> p t h", h=nheads),
                            op=OP.add)
                        nc.vector.scalar_tensor_tensor(
                            out=s_t[:, 0:nb_t * nheads],
                            in0=s_t[:, 0:nb_t * nheads], scalar=NEG_SLOPE,
                            in1=s_t[:, 0:nb_t * nheads],
                            op0=OP.mult, op1=OP.max)
                        nc.scalar.activation(
                            rhs[:, 0:nb_t, hcols:rhsw],
                            s_t[:, 0:nb_t * nheads].rearrange(
                                "p (t h) -> p t h", h=nheads),
                            AF.Exp)
                        nc.vector.tensor_tensor(
                            out=rhs[:, 0:nb_t, 0:hcols],
                            in0=G[:, 0:nb_t, 0:hcols],
                            in1=rhs[:, 0:nb_t, hcols:rhsw].unsqueeze(
                                3).to_broadcast([P, nb_t, nheads, hdim]),
                            op=OP.mult)
                        for k in range(nb_t):
                            nc.tensor.matmul(
                                out=acc[:], lhsT=oh[:, k, :], rhs=rhs[:, k, :],
                                start=(t + k == t0), stop=(t + k == t1 - 1),
                                skip_group_check=True)
                        t += nb_t
                    close_fn(b, acc)

            # ---- L1 close: normalize + bias + elu -> h1_own
            h1 = pp.tile([P, nb, hh], F32)
            nc.vector.memset(h1[:, nb - 1, :], 0.0)

            def close1(b, acc):
                d8 = wp.tile([P, heads], F32, tag="d8")
                nc.vector.tensor_scalar(out=d8[:], in0=acc[:, hh:hh + heads],
                                        scalar1=1e-16, scalar2=None, op0=OP.add)
                r8 = wp.tile([P, heads], F32, tag="r8")
                nc.vector.reciprocal(r8[:], d8[:])
                tt = wp.tile([P, hh], F32, tag="tt")
                nc.vector.tensor_tensor(
                    out=tt[:].rearrange("p (h c) -> p h c", h=heads),
                    in0=acc[:, 0:hh].rearrange("p (h c) -> p h c", h=heads),
                    in1=r8[:].unsqueeze(2).to_broadcast([P, heads, nhid]),
                    op=OP.mult)
                nc.vector.tensor_tensor(out=tt[:], in0=tt[:], in1=b1_t[:],
                                        op=OP.add)
                pos = wp.tile([P, hh], F32, tag="pos")
                neg = wp.tile([P, hh], F32, tag="neg")
                nc.vector.tensor_scalar(out=pos[:], in0=tt[:], scalar1=0.0,
                                        scalar2=None, op0=OP.max)
                nc.vector.tensor_scalar(out=neg[:], in0=tt[:], scalar1=0.0,
                                        scalar2=None, op0=OP.min)
                nc.scalar.activation(neg[:], neg[:], AF.Exp)
                nc.vector.scalar_tensor_tensor(
                    out=h1[:, b, :], in0=pos[:], scalar=-1.0, in1=neg[:],
                    op0=OP.add, op1=OP.add)

            if do_l1:
                edge_layer(table1, d1, hh, heads, nhid, adst1_own, close1)
            else:
                nc.vector.memset(h1[:], 0.0)

            # ---- phase C: W2_ext, h2_ext shard, allgather table2
            w2_t = wp.tile([hh, nclass], F32, tag="w2")
            nc.sync.dma_start(w2_t[:], W2[:])
            as2 = wp.tile([P, nclass], F32, tag="as2")
            ad2 = wp.tile([P, nclass], F32, tag="ad2")
            nc.sync.dma_start(as2[:], asrc2[:])
            nc.sync.dma_start(ad2[:], adst2[:])
            w2e = pp.tile([hh, d2], F32)
            nc.scalar.copy(w2e[:, 0:nclass], w2_t[:])
            tmp2 = wp.tile([P, nclass], F32, tag="tmp2")
            nc.vector.tensor_tensor(out=tmp2[:], in0=w2_t[:], in1=as2[:], op=OP.mult)
            nc.vector.tensor_reduce(out=w2e[:, nclass:nclass + 1], in_=tmp2[:],
                                    axis=mybir.AxisListType.X, op=OP.add)
            nc.vector.tensor_tensor(out=tmp2[:], in0=w2_t[:], in1=ad2[:], op=OP.mult)
            nc.vector.tensor_reduce(out=w2e[:, nclass + 1:d2], in_=tmp2[:],
                                    axis=mybir.AxisListType.X, op=OP.add)

            adst2_own = pp.tile([P, nb, 1], F16)
            nc.vector.memset(adst2_own[:], 0.0)
            for b in range(nb):
                n0 = b * P
                cnt = min(P, npc - n0)
                tps = psA.tile([P, P], F32, tag="ps_a")
                nc.tensor.transpose(out=tps[:], in_=h1[:, b, :], identity=ident[:])
                h1T = wp.tile([P, P], F32, tag="h1T")
                nc.scalar.copy(h1T[:], tps[:])
                ps2 = psA.tile([P, d2], F32, tag="ps_a")
                nc.tensor.matmul(out=ps2[:cnt, :], lhsT=h1T[:, 0:cnt], rhs=w2e[:],
                                 start=True, stop=True)
                stg2 = wp.tile([P, d2], F16, tag="stg2")
                if cnt < P:
                    nc.vector.memset(stg2[:], 0.0)
                nc.scalar.copy(stg2[:cnt, :], ps2[:cnt, :])
                nc.vector.tensor_copy(adst2_own[:cnt, b, :],
                                      stg2[:cnt, nclass + 1:d2])
                nc.sync.dma_start(shard2[n0:n0 + P, :], stg2[:])
            if do_cc:
                nc.gpsimd.collective_compute(
                    "AllGather", OP.bypass,
                    replica_groups=[list(range(N_CORES))],
                    ins=[shard2[:]], outs=[table2[:]])
            else:
                nc.sync.dma_start(table2[0:npc_pad, :], shard2[:])

            # ---- L2 close: log_softmax -> out
            def close2(b, acc):
                n0 = b * P
                cnt = min(P, npc - n0)
                d1_ = wp.tile([P, 1], F32, tag="d1_")
                nc.vector.tensor_scalar(out=d1_[:], in0=acc[:, nclass:nclass + 1],
                                        scalar1=1e-16, scalar2=None, op0=OP.add)
                r1 = wp.tile([P, 1], F32, tag="r1")
                nc.vector.reciprocal(r1[:], d1_[:])
                z = wp.tile([P, nclass], F32, tag="z")
                nc.vector.tensor_scalar(out=z[:], in0=acc[:, 0:nclass],
                                        scalar1=r1[:, 0:1], scalar2=None,
                                        op0=OP.mult)
                nc.vector.tensor_tensor(out=z[:], in0=z[:], in1=b2_t[:], op=OP.add)
                m = wp.tile([P, 1], F32, tag="m")
                nc.vector.tensor_reduce(out=m[:], in_=z[:],
                                        axis=mybir.AxisListType.X, op=OP.max)
                nc.vector.tensor_scalar(out=z[:], in0=z[:], scalar1=m[:, 0:1],
                                        scalar2=None, op0=OP.subtract)
                e = wp.tile([P, nclass], F32, tag="e")
                se = wp.tile([P, 1], F32, tag="se")
                nc.scalar.activation(e[:], z[:], AF.Exp, accum_out=se[:])
                lse = wp.tile([P, 1], F32, tag="lse")
                nc.scalar.activation(lse[:], se[:], AF.Ln)
                ob = wp.tile([P, nclass], F32, tag="ob")
                nc.vector.tensor_scalar(out=ob[:], in0=z[:], scalar1=lse[:, 0:1],
                                        scalar2=None, op0=OP.subtract)
                nc.sync.dma_start(out[n0:n0 + cnt, :], ob[:cnt, :])

            if do_l2:
                edge_layer(table2, d2, nclass, 1, nclass, adst2_own, close2)
            else:
                for b in range(nb):
                    n0 = b * P
                    cnt = min(P, npc - n0)
                    zb = wp.tile([P, nclass], F32, tag="zb")
                    nc.vector.memset(zb[:], 0.0)
                    nc.sync.dma_start(out[n0:n0 + cnt, :], zb[:cnt, :])

    return nc


_CACHE = {}


def _get_program(bake, nfeat, nhid, heads, nclass, do_l1=True, do_l2=True, do_cc=True):
    key = (bake["nt"], tuple(bake["tiles_per_block"]), nfeat, nhid, heads, nclass,
           do_l1, do_l2, do_cc, G_BUFS, SCRATCH)
    if key not in _CACHE:
        nc = _build_program(bake, nfeat, nhid, heads, nclass, do_l1, do_l2, do_cc)
        _split_multi_waits(nc)
        _CACHE[key] = nc
    return _CACHE[key]


def kernel(x, edge_index, W1, att_src1, att_dst1, b1, W2, att_src2, att_dst2, b2):
    from concourse.bass_utils import run_bass_kernel_spmd
    n_nodes, nfeat = x.shape
    heads, nhid = att_src1.shape[1], att_src1.shape[2]
    nclass = att_src2.shape[2]
    npc = n_nodes // N_CORES

    bake, src_idx_all, dst_loc_all = _host_prep(np.asarray(edge_index), n_nodes)
    nc = _get_program(bake, nfeat, nhid, heads, nclass)

    x = np.asarray(x, np.float32)
    in_maps = []
    for c in range(N_CORES):
        in_maps.append({
            "xT": np.ascontiguousarray(x[c * npc:(c + 1) * npc].T),
            "W1": np.asarray(W1, np.float32),
            "W2": np.asarray(W2, np.float32),
            "asrc1": np.tile(np.asarray(att_src1, np.float32).reshape(1, -1), (P, 1)),
            "adst1": np.tile(np.asarray(att_dst1, np.float32).reshape(1, -1), (P, 1)),
            "asrc2": np.tile(np.asarray(att_src2, np.float32).reshape(1, -1), (P, 1)),
            "adst2": np.tile(np.asarray(att_dst2, np.float32).reshape(1, -1), (P, 1)),
            "b1r": np.tile(np.asarray(b1, np.float32).reshape(1, -1), (P, 1)),
            "b2r": np.tile(np.asarray(b2, np.float32).reshape(1, -1), (P, 1)),
            "srcidx": src_idx_all[c],
            "iota128": np.tile(np.arange(P, dtype=np.float32), (P, 1)),
            "ident128": np.eye(P, dtype=np.float32),
            "dstloc": dst_loc_all[c],
        })
    res = run_bass_kernel_spmd(nc, in_maps, core_ids=list(range(N_CORES)))
    return np.concatenate([res.results[c]["out"] for c in range(N_CORES)], axis=0)
